# revision 1
# baseline (speedup 1.0000x reference)
"""Distributed Trainium2 kernel for relative-position causal attention.

N=M=2048, B=1, D=1024, H=16, DQK=DV=64, OFFSET=0.
2 heads per core on 8 NeuronCores. Per core:
  - projections: Q/K/PErev in transposed layout [dqk2, n], V natural [m, dv]
  - attention over row-block PAIRS (2j, 2j+1): S = Q K^T + skew(Q PErev^T);
    the rel-shift gather is an SBUF->SBUF DMA whose source row stride is
    (width-1) i.e. a diagonal read; the causal mask is the -1e30 pad the
    diagonal tile's upper triangle reads
  - exp without max subtraction (logits bounded); P^T via PE matmuls against
    diag(1/l) which transposes and normalizes at once; pair processing makes
    each ctx matmul cover 256 output columns
  - two AllToAlls ship ctx^T halves; each core computes a 256-row slice of
    out = ctx @ to_out^T; host concatenates slices.
"""

import sys

sys.path.insert(0, "/opt/trn_rl_repo")

import numpy as np
import ml_dtypes

from concourse import bass, bacc, tile, mybir
from concourse.ap import AP
from concourse.bass_utils import run_bass_kernel_spmd

N, M, D, H, DQK, DV = 2048, 2048, 1024, 16, 64, 64
RP = 2048
NCORES = 8
NB = N // 128
KT = D // 128
F2 = RP + 128
SLICE = N // NCORES

BF = mybir.dt.bfloat16
F32 = mybir.dt.float32
RG = [list(range(NCORES))]
Copy = None  # set at build time
_cache = {}


def _build():
    nc = bacc.Bacc("TRN2", target_bir_lowering=False, debug=False,
                   num_devices=NCORES)
    ACTF = mybir.ActivationFunctionType

    xqT = nc.dram_tensor("xqT", [D, N], BF, kind="ExternalInput")
    xkvT = nc.dram_tensor("xkvT", [D, M], BF, kind="ExternalInput")
    scT = nc.dram_tensor("scT", [D, RP], BF, kind="ExternalInput")
    wqT = nc.dram_tensor("wqT", [128, KT * 128], BF, kind="ExternalInput")
    wkT = nc.dram_tensor("wkT", [128, KT * 128], BF, kind="ExternalInput")
    wvT = nc.dram_tensor("wvT", [128, KT * 128], BF, kind="ExternalInput")
    fpeT = nc.dram_tensor("fpeT", [128, KT * 128], BF, kind="ExternalInput")
    woT = nc.dram_tensor("woT", [128, KT * D], BF, kind="ExternalInput")
    identc = nc.dram_tensor("identc", [128, 128], BF, kind="ExternalInput")
    out_ext = nc.dram_tensor("out", [SLICE, D], F32, kind="ExternalOutput")

    cc_in = [nc.dram_tensor(f"cc_in{h}", [128 * NCORES, 128], BF)
             for h in range(2)]
    cc_out = [nc.dram_tensor(f"cc_out{h}", [128 * NCORES, 128], BF)
              for h in range(2)]

    with tile.TileContext(nc) as tc:
        with (
            tc.tile_pool(name="const", bufs=1) as cpool,
            tc.tile_pool(name="proj", bufs=1) as proj,
            tc.tile_pool(name="xstream", bufs=3) as xstream,
            tc.tile_pool(name="work", bufs=3) as work,
            tc.tile_pool(name="small", bufs=4) as small,
        ):
            # ---- constants (pre-shuffled on host for contiguous DMA)
            wq_sb = cpool.tile([128, KT, 128], BF, tag="wq")
            wk_sb = cpool.tile([128, KT, 128], BF, tag="wk")
            wv_sb = cpool.tile([128, KT, 128], BF, tag="wv")
            fpe_sb = cpool.tile([128, KT, 128], BF, tag="fpe")
            for dst, srcw in ((wq_sb, wqT), (wk_sb, wkT), (wv_sb, wvT),
                              (fpe_sb, fpeT)):
                nc.gpsimd.dma_start(
                    dst[:], srcw.ap().rearrange("p (k c) -> p k c", k=KT))
            wo_sb = cpool.tile([128, KT, D], BF, tag="wo")
            nc.gpsimd.dma_start(
                wo_sb[:], woT.ap().rearrange("p (k c) -> p k c", k=KT))
            ident = cpool.tile([128, 128], BF, tag="ident")
            nc.gpsimd.dma_start(ident[:], identc[:])

            # ---- persistent activations
            q2T = proj.tile([128, N], BF, tag="q2T")
            k2T = proj.tile([128, M], BF, tag="k2T")
            pe2T = proj.tile([128, RP], BF, tag="pe2T")
            v2 = proj.tile([128, NB, 128], BF, tag="v2")
            ctxh = [[proj.tile([64, N // 2], BF, tag=f"ctxh{h}_{p}",
                               name=f"ctxh{h}_{p}") for p in range(2)]
                    for h in range(2)]
            plrbuf = [proj.tile([128, F2], F32, tag=f"plr{i}", name=f"plr{i}")
                      for i in range(4)]
            for i in range(4):
                nc.vector.memset(plrbuf[i][:, RP:F2], -1e30)

            # ---- projections (scoped pools; freed before attention)
            with (
                tc.tile_pool(name="xkvp", bufs=1) as xkvp,
                tc.tile_pool(name="psP", bufs=1, space="PSUM") as psP,
            ):
                xkv_t = []
                for k in range(KT):
                    t = xkvp.tile([128, M], BF, tag=f"xkv{k}")
                    nc.scalar.dma_start(t[:], xkvT[k * 128:(k + 1) * 128, :])
                    xkv_t.append(t)
                # K
                for ch in range(M // 512):
                    ps = psP.tile([128, 512], F32, tag=f"hacc{ch}",
                                  name=f"kacc{ch}")
                    for k in range(KT):
                        nc.tensor.matmul(ps[:], wk_sb[:, k, :],
                                         xkv_t[k][:, ch * 512:(ch + 1) * 512],
                                         start=(k == 0), stop=(k == KT - 1))
                    nc.scalar.activation(k2T[:, ch * 512:(ch + 1) * 512],
                                         ps[:], ACTF.Copy)
                # Q and PErev (streamed); PE chunks high-to-low
                for which, xdram, wtile, dest, chorder in (
                    ("q", xqT, wq_sb, q2T, (0, 1, 2, 3)),
                    ("pe", scT, fpe_sb, pe2T, (3, 2, 1, 0)),
                ):
                    accs = {ch: psP.tile([128, 512], F32, tag=f"hacc{ch}",
                                         name=f"{which}acc{ch}")
                            for ch in chorder}
                    for k in range(KT):
                        t = xstream.tile([128, N], BF, tag="xs",
                                         name=f"{which}x{k}")
                        nc.sync.dma_start(t[:],
                                          xdram[k * 128:(k + 1) * 128, :])
                        for ch in chorder:
                            nc.tensor.matmul(
                                accs[ch][:], wtile[:, k, :],
                                t[:, ch * 512:(ch + 1) * 512],
                                start=(k == 0), stop=(k == KT - 1))
                    for ch in chorder:
                        nc.scalar.activation(
                            dest[:, ch * 512:(ch + 1) * 512], accs[ch][:],
                            ACTF.Copy)
                # V natural layout
                for mt in range(NB):
                    ps = psP.tile([128, 128], F32, tag="projv", bufs=2,
                                  name=f"vacc{mt}")
                    for k in range(KT):
                        nc.tensor.matmul(
                            ps[:], xkv_t[k][:, mt * 128:(mt + 1) * 128],
                            wv_sb[:, k, :],
                            start=(k == 0), stop=(k == KT - 1))
                    nc.scalar.activation(v2[:, mt, :], ps[:], ACTF.Copy)

            # ---- attention over block pairs (A=2j, B=2j+1)
            with (
                tc.tile_pool(name="psS", bufs=3, space="PSUM") as psS,
                tc.tile_pool(name="psT", bufs=3, space="PSUM") as psT,
                tc.tile_pool(name="psX", bufs=2, space="PSUM") as psX,
            ):
                psR = psS
                cp = [0]  # copy-engine round robin

                def copy(dst, srcp):
                    eng = (nc.vector, nc.scalar)[cp[0] & 1]
                    cp[0] += 1
                    if eng is nc.scalar:
                        nc.scalar.activation(dst, srcp, ACTF.Copy)
                    else:
                        nc.vector.tensor_copy(dst, srcp)

                for j in range(NB // 2):
                    for hl in range(2):
                        hb = hl * 64
                        blk = {}
                        for g, nb in ((0, 2 * j), (1, 2 * j + 1)):
                            n0 = nb * 128
                            span = n0 + 128
                            c_lo = (RP - 1 - n0 - 127) // 512
                            plr = plrbuf[(j * 4 + hl * 2 + g) % 4]
                            for ch in range(c_lo, RP // 512):
                                ps = psR.tile([128, 512], F32, tag="cont")
                                nc.tensor.matmul(
                                    ps[:],
                                    q2T[hb:hb + 64, n0:n0 + 128],
                                    pe2T[hb:hb + 64,
                                         ch * 512:(ch + 1) * 512],
                                    start=True, stop=True)
                                copy(plr[:, ch * 512:(ch + 1) * 512], ps[:])
                            sS = work.tile([128, span], F32, tag="sS",
                                           name=f"sS{g}")
                            skew = AP(plr[:].tensor,
                                      plr[:].offset + (RP - 1 - n0),
                                      [[F2 - 1, 128], [1, span]])
                            nc.gpsimd.dma_start(sS[:], skew)
                            for ch in range((span + 511) // 512):
                                cw = min(512, span - ch * 512)
                                ps = psS.tile([128, 512], F32, tag="cont")
                                nc.tensor.matmul(
                                    ps[:, :cw],
                                    q2T[hb:hb + 64, n0:n0 + 128],
                                    k2T[hb:hb + 64, ch * 512:ch * 512 + cw],
                                    start=True, stop=True)
                                nc.vector.tensor_tensor(
                                    sS[:, ch * 512:ch * 512 + cw],
                                    ps[:, :cw],
                                    sS[:, ch * 512:ch * 512 + cw],
                                    mybir.AluOpType.add)
                            pP = work.tile([128, span], BF, tag="pP",
                                           name=f"pP{g}")
                            nche = (span + 511) // 512
                            lrow4 = small.tile([128, nche], F32, tag="lrow4",
                                               name=f"lrow4_{g}")
                            for ch in range(nche):
                                cw = min(512, span - ch * 512)
                                nc.scalar.activation(
                                    pP[:, ch * 512:ch * 512 + cw],
                                    sS[:, ch * 512:ch * 512 + cw], ACTF.Exp,
                                    accum_out=lrow4[:, ch:ch + 1])
                            linv = small.tile([128, 1], F32, tag="linv")
                            if nche > 1:
                                lrow = small.tile([128, 1], F32, tag="lrow")
                                nc.vector.tensor_reduce(
                                    lrow[:], lrow4[:],
                                    mybir.AxisListType.X,
                                    mybir.AluOpType.add)
                                nc.vector.reciprocal(linv[:], lrow[:])
                            else:
                                nc.vector.reciprocal(linv[:], lrow4[:])
                            diagt = small.tile([128, 128], BF, tag="diagt",
                                               name=f"diagt{g}")
                            nc.vector.tensor_scalar_mul(diagt[:], ident[:],
                                                        linv[:])
                            blk[g] = (pP, diagt)
                        # P^T tiles: batch A[mt] and B[mt] into one PSUM
                        # bank -> one wide copy -> one [64, 256] ctx matmul
                        ctxp = psX.tile([64, 256], F32, tag="ctx")
                        na, nbt = 2 * j + 1, 2 * j + 2  # tile counts A, B
                        for g0 in range(0, nbt, 2):
                            pt_ps = psT.tile([128, 512], F32, tag="ptT")
                            pt_sb = small.tile([128, 512], BF, tag="ptsb")
                            for q, mt in enumerate(range(g0,
                                                         min(g0 + 2, nbt))):
                                if mt < na:
                                    nc.tensor.matmul(
                                        pt_ps[:, q * 256:q * 256 + 128],
                                        blk[0][0][:,
                                                  mt * 128:(mt + 1) * 128],
                                        blk[0][1][:], start=True, stop=True)
                                nc.tensor.matmul(
                                    pt_ps[:, q * 256 + 128:q * 256 + 256],
                                    blk[1][0][:, mt * 128:(mt + 1) * 128],
                                    blk[1][1][:], start=True, stop=True)
                            w = min(2, nbt - g0) * 256
                            copy(pt_sb[:, :w], pt_ps[:, :w])
                            for q, mt in enumerate(range(g0,
                                                         min(g0 + 2, nbt))):
                                if mt < na:
                                    nc.tensor.matmul(
                                        ctxp[:],
                                        v2[:, mt, hl * 64:hl * 64 + 64],
                                        pt_sb[:, q * 256:(q + 1) * 256],
                                        start=(mt == 0),
                                        stop=(mt == nbt - 1))
                                else:
                                    nc.tensor.matmul(
                                        ctxp[:, 128:256],
                                        v2[:, mt, hl * 64:hl * 64 + 64],
                                        pt_sb[:, q * 256 + 128:
                                              q * 256 + 256],
                                        start=False, stop=(mt == nbt - 1))
                        copy(ctxh[hl][0][:, j * 128:(j + 1) * 128],
                             ctxp[:, 0:128])
                        copy(ctxh[hl][1][:, j * 128:(j + 1) * 128],
                             ctxp[:, 128:256])

                # ---- ship halves: cc_in writes + AllToAll
                for half in range(2):
                    cc_i = cc_in[half]
                    for hl in range(2):
                        dst = AP(cc_i, hl * 64 * 128,
                                 [[128, 64], [128 * 128, NCORES], [1, 128]])
                        s = ctxh[hl][half][:]
                        s = AP(s.tensor, s.offset,
                               [[N // 2, 64], [128, NCORES], [1, 128]])
                        nc.sync.dma_start(dst, s)
                    nc.gpsimd.collective_compute(
                        "AllToAll",
                        mybir.AluOpType.bypass,
                        ins=[cc_i[:]],
                        outs=[cc_out[half][:]],
                        replica_groups=RG,
                    )

            # ---- out projection per half
            with tc.tile_pool(name="psO", bufs=2, space="PSUM") as psO:
                for half in range(2):
                    stages = [small.tile([128, 128], BF, tag=f"ccst{k % 2}",
                                         name=f"ccst{half}_{k}")
                              for k in range(KT)]
                    for k in range(KT):
                        nc.sync.dma_start(
                            stages[k][:],
                            cc_out[half][k * 128:(k + 1) * 128, :])
                    for dc in range(D // 512):
                        ps = psO.tile([128, 512], F32, tag="out")
                        for k in range(KT):
                            nc.tensor.matmul(
                                ps[:],
                                stages[k][:],
                                wo_sb[:, k, dc * 512:(dc + 1) * 512],
                                start=(k == 0), stop=(k == KT - 1))
                        ostage = small.tile([128, 512], F32, tag="ostage")
                        nc.scalar.activation(ostage[:], ps[:], ACTF.Copy)
                        nc.sync.dma_start(
                            out_ext[half * 128:(half + 1) * 128,
                                    dc * 512:(dc + 1) * 512], ostage[:])

    nc.compile()
    return nc


def _host_prep(inputs):
    bf16 = ml_dtypes.bfloat16
    x_q = np.asarray(inputs["x_q"])[:, 0, :]
    x_kv = np.asarray(inputs["x_kv"])[:, 0, :]
    to_q = np.asarray(inputs["to_q"])
    to_k = np.asarray(inputs["to_k"])
    to_v = np.asarray(inputs["to_v"])
    to_out = np.asarray(inputs["to_out"])
    fpe = np.asarray(inputs["for_pos_enc"])

    xqT = np.ascontiguousarray(x_q.T).astype(bf16)
    xkvT = np.ascontiguousarray(x_kv.T).astype(bf16)

    r = np.arange(0, RP, dtype=np.float32)
    inv_freq = 1.0 / (10000.0 ** (np.arange(0.0, D, 2.0, np.float32) / D))
    ph = r[:, None] * inv_freq[None, :]
    sincos = np.concatenate([np.sin(ph), np.cos(ph)], axis=-1)
    scT = np.ascontiguousarray(sincos[::-1].T).astype(bf16)

    wo_ckd = (to_out.transpose(0, 2, 1).reshape(D, H * DV).T
              .reshape(KT, 128, D).transpose(1, 0, 2).reshape(128, KT * D))
    woT = np.ascontiguousarray(wo_ckd).astype(bf16)
    identity = np.eye(128, dtype=bf16)

    def shuf(w):
        # [D, 128] -> [p, k*c] with the k-tile index on the free axis
        return np.ascontiguousarray(
            w.reshape(KT, 128, 128).transpose(1, 0, 2).reshape(128, KT * 128)
        ).astype(bf16)

    in_maps = []
    for c in range(NCORES):
        hs = [2 * c, 2 * c + 1]
        in_maps.append({
            "xqT": xqT, "xkvT": xkvT, "scT": scT,
            "wqT": shuf(np.concatenate([to_q[:, h, :].T for h in hs], 1)),
            "wkT": shuf(np.concatenate([to_k[:, h, :].T for h in hs], 1)),
            "wvT": shuf(np.concatenate([to_v[:, h, :].T for h in hs], 1)),
            "fpeT": shuf(np.concatenate([fpe[:, h, :].T for h in hs], 1)),
            "woT": woT, "identc": identity,
        })
    return in_maps


def kernel(**inputs):
    if "nc" not in _cache:
        _cache["nc"] = _build()
    nc = _cache["nc"]
    in_maps = _host_prep(inputs)
    res = run_bass_kernel_spmd(nc, in_maps, list(range(NCORES)))
    out = np.concatenate([res.results[c]["out"] for c in range(NCORES)], 0)
    return out.reshape(N, 1, D).astype(np.float32)


if __name__ == "__main__":
    import pickle
    with open("/tmp/inputs.pkl", "rb") as f:
        inputs = pickle.load(f)
    out = kernel(**inputs)
    exp = np.load("/tmp/expected.npy")
    err = np.linalg.norm(out - exp) / np.linalg.norm(exp)
    print("Relative error:", err)

